# revision 1
# baseline (speedup 1.0000x reference)
"""Trainium2 Bass kernel for nn_GNN_Model (gnn_message_passing).

Data-parallel over B=16384 (query,mv) pairs across 8 cores (2048 each).
Per core, feature-major pipeline:
  gather nbr rows (indirect DMA, 512B rows) -> PE transpose -> fp16 Xt
  -> stationary-weight gate matmuls (f,i) -> ACT sigmoid(+bias) / tanh
  -> VE products + segmented free-axis reduce (mean over 32 neighbors)
  -> query/mv tail (Wo, Wmv, W1, W2) all feature-major, biases folded.
"""

import os

import numpy as np

import concourse.bass as bass
import concourse.mybir as mybir
import concourse.tile as tile
from concourse import bacc
from concourse.bass_utils import run_bass_kernel_spmd
from concourse.masks import make_identity

N = 500000
D = 128
H = 256
B = 16384
KN = 32
NCORES = 8
BC = B // NCORES          # 2048 rows per core
GROUP = 2048              # rows per processing group (= 16 chunks of 128)
NGR = (BC * KN) // GROUP  # 32 neighbor groups
NCH = BC * KN // 128      # 512 neighbor chunks
NCHT = NCH + 2 * (BC // 128)  # + 16 query chunks + 16 mv chunks = 544
FP16 = mybir.dt.float16
F32 = mybir.dt.float32
LAST_EXEC_NS = None


def _build(b2_imm: float):
    nc = bacc.Bacc(None, target_bir_lowering=False)

    feats = nc.dram_tensor("feats", [N, D], F32, kind="ExternalInput")
    idx = nc.dram_tensor("idx", [128, NCHT], mybir.dt.int32, kind="ExternalInput")
    w_names = ["wf", "wi", "wo", "wmva", "wmvb", "w1qa", "w1qb", "w1ma", "w1mb"]
    wt = {n: nc.dram_tensor(n, [128, 128], FP16, kind="ExternalInput") for n in w_names}
    wt["w2a"] = nc.dram_tensor("w2a", [128, 1], FP16, kind="ExternalInput")
    wt["w2b"] = nc.dram_tensor("w2b", [128, 1], FP16, kind="ExternalInput")
    b_names = ["bf", "bi", "bo", "b1a", "b1b"]
    bt = {n: nc.dram_tensor(n, [128, 1], F32, kind="ExternalInput") for n in b_names}
    out = nc.dram_tensor("out", [1, BC], F32, kind="ExternalOutput")

    with tile.TileContext(nc) as tc:
        with (
            tc.tile_pool(name="const", bufs=1) as cp,
            tc.tile_pool(name="stage", bufs=8) as stp,
            tc.tile_pool(name="xt", bufs=2) as xtp,
            tc.tile_pool(name="gate", bufs=2) as gp,
            tc.tile_pool(name="ve", bufs=2) as vp,
            tc.tile_pool(name="xtps", bufs=2, space="PSUM") as xtpp,
            tc.tile_pool(name="gps", bufs=1, space="PSUM") as gpp,
        ):
            ident = cp.tile([128, 128], F32)
            make_identity(nc, ident[:])
            idx_t = cp.tile([128, NCHT], mybir.dt.int32)
            nc.sync.dma_start(out=idx_t[:], in_=idx[:])
            w = {}
            for n, dr in wt.items():
                w[n] = cp.tile([128, dr.shape[1]], FP16, tag=f"w_{n}", name=f"w_{n}")
                nc.sync.dma_start(out=w[n][:], in_=dr[:])
            bias = {}
            for n, dr in bt.items():
                bias[n] = cp.tile([128, 1], F32, tag=f"b_{n}", name=f"b_{n}")
                nc.sync.dma_start(out=bias[n][:], in_=dr[:])
            c_sum = cp.tile([128, BC], F32)      # c.T (unscaled sum over k)
            qt_sb = cp.tile([128, BC], FP16)     # query feats transposed
            mvt_sb = cp.tile([128, BC], FP16)

            def gather_transpose_group(ch0, dest_sb):
                # chunks ch0..ch0+15 (128 rows each) -> dest_sb[:, 0:2048] fp16, transposed
                for s in range(4):
                    xt_ps = xtpp.tile([128, 512], F32, tag="xtps")
                    for t4 in range(4):
                        ch = ch0 + s * 4 + t4
                        stage = stp.tile([128, D], F32, tag="stage")
                        nc.gpsimd.indirect_dma_start(
                            out=stage[:],
                            out_offset=None,
                            in_=feats[:],
                            in_offset=bass.IndirectOffsetOnAxis(
                                ap=idx_t[:, ch:ch + 1], axis=0
                            ),
                        )
                        nc.tensor.transpose(
                            xt_ps[:, t4 * 128:(t4 + 1) * 128], stage[:], ident[:]
                        )
                    nc.vector.tensor_copy(
                        out=dest_sb[:, s * 512:(s + 1) * 512],
                        in_=xt_ps[:],
                    )

            SIG = mybir.ActivationFunctionType.Sigmoid
            TANH = mybir.ActivationFunctionType.Tanh
            RELU = mybir.ActivationFunctionType.Relu
            COPY = mybir.ActivationFunctionType.Copy
            MUL = mybir.AluOpType.mult

            # ---- main neighbor loop ----
            for g in range(NGR):
                xt_sb = xtp.tile([128, GROUP], FP16, tag="xt")
                gather_transpose_group(g * 16, xt_sb)
                f_sb = gp.tile([128, GROUP], FP16, tag="f")
                i_sb = gp.tile([128, GROUP], FP16, tag="i")
                t_sb = gp.tile([128, GROUP], FP16, tag="t")
                for gb in range(2):  # gate batches of 1024 rows
                    f_ps = gpp.tile([128, 1024], F32, tag="fps")
                    i_ps = gpp.tile([128, 1024], F32, tag="ips")
                    for s2 in range(2):
                        rhs = xt_sb[:, (gb * 2 + s2) * 512:(gb * 2 + s2 + 1) * 512]
                        nc.tensor.matmul(f_ps[:, s2 * 512:(s2 + 1) * 512],
                                         lhsT=w["wf"][:], rhs=rhs, start=True, stop=True)
                        nc.tensor.matmul(i_ps[:, s2 * 512:(s2 + 1) * 512],
                                         lhsT=w["wi"][:], rhs=rhs, start=True, stop=True)
                    nc.scalar.activation(f_sb[:, gb * 1024:(gb + 1) * 1024], f_ps[:],
                                         SIG, bias=bias["bf"][:])
                    nc.scalar.activation(i_sb[:, gb * 1024:(gb + 1) * 1024], i_ps[:],
                                         SIG, bias=bias["bi"][:])
                nc.scalar.activation(t_sb[:], xt_sb[:], TANH)
                fi = vp.tile([128, GROUP], FP16, tag="fi")
                prod = vp.tile([128, GROUP], FP16, tag="prod")
                nc.vector.tensor_tensor(out=fi[:], in0=f_sb[:], in1=i_sb[:], op=MUL)
                nc.vector.tensor_tensor(out=prod[:], in0=fi[:], in1=t_sb[:], op=MUL)
                nc.vector.tensor_reduce(
                    out=c_sum[:, g * (GROUP // KN):(g + 1) * (GROUP // KN)],
                    in_=prod[:].rearrange("p (b k) -> p b k", k=KN),
                    axis=mybir.AxisListType.X,
                    op=mybir.AluOpType.add,
                )

            # ---- query/mv transposes ----
            gather_transpose_group(NCH, qt_sb)
            gather_transpose_group(NCH + 16, mvt_sb)

            # tanh(c/KN)
            tc_sb = cp.tile([128, BC], FP16)
            nc.scalar.activation(tc_sb[:], c_sum[:], TANH, scale=1.0 / KN)

            # ---- per-source embedding (Wo gate + Wmv projection) ----
            emb = {}
            for src_name, src_t in (("q", qt_sb), ("mv", mvt_sb)):
                emb_sb = cp.tile([128, BC], FP16, tag=f"emb_{src_name}",
                                 name=f"emb_{src_name}")
                for hh in range(2):  # halves of 1024 cols
                    cols = slice(hh * 1024, (hh + 1) * 1024)
                    o_ps = gpp.tile([128, 1024], F32, tag="fps")
                    for s2 in range(2):
                        c0 = hh * 1024 + s2 * 512
                        nc.tensor.matmul(o_ps[:, s2 * 512:(s2 + 1) * 512],
                                         lhsT=w["wo"][:], rhs=src_t[:, c0:c0 + 512],
                                         start=True, stop=True)
                    o_sb = vp.tile([128, 1024], FP16, tag="o")
                    nc.scalar.activation(o_sb[:], o_ps[:], SIG, bias=bias["bo"][:])
                    h_sb = vp.tile([128, 1024], FP16, tag="h")
                    nc.vector.tensor_tensor(out=h_sb[:], in0=o_sb[:],
                                            in1=tc_sb[:, cols], op=MUL)
                    e_ps = gpp.tile([128, 1024], F32, tag="ips")
                    for s2 in range(2):
                        c0 = hh * 1024 + s2 * 512
                        sl = slice(s2 * 512, (s2 + 1) * 512)
                        nc.tensor.matmul(e_ps[:, sl], lhsT=w["wmva"][:],
                                         rhs=src_t[:, c0:c0 + 512], start=True, stop=False)
                        nc.tensor.matmul(e_ps[:, sl], lhsT=w["wmvb"][:],
                                         rhs=h_sb[:, sl], start=False, stop=True)
                    nc.scalar.activation(emb_sb[:, cols], e_ps[:], COPY)
                emb[src_name] = emb_sb

            # ---- final MLP ----
            hid = [cp.tile([128, BC], FP16, tag=f"hid{h}", name=f"hid{h}")
                   for h in range(2)]
            for hh in range(2):      # hidden fo halves
                wq = w["w1qa"] if hh == 0 else w["w1qb"]
                wm = w["w1ma"] if hh == 0 else w["w1mb"]
                b1 = bias["b1a"] if hh == 0 else bias["b1b"]
                for ch in range(2):  # col halves
                    h_ps = gpp.tile([128, 1024], F32, tag="fps")
                    for s2 in range(2):
                        c0 = ch * 1024 + s2 * 512
                        sl = slice(s2 * 512, (s2 + 1) * 512)
                        nc.tensor.matmul(h_ps[:, sl], lhsT=wq[:],
                                         rhs=emb["q"][:, c0:c0 + 512], start=True, stop=False)
                        nc.tensor.matmul(h_ps[:, sl], lhsT=wm[:],
                                         rhs=emb["mv"][:, c0:c0 + 512], start=False, stop=True)
                    nc.scalar.activation(hid[hh][:, ch * 1024:(ch + 1) * 1024],
                                         h_ps[:], RELU, bias=b1[:])
            ben_sb = cp.tile([1, BC], F32)
            for ch in range(2):
                b_ps = gpp.tile([1, 1024], F32, tag="bps")
                for s2 in range(2):
                    c0 = ch * 1024 + s2 * 512
                    sl = slice(s2 * 512, (s2 + 1) * 512)
                    nc.tensor.matmul(b_ps[:, sl], lhsT=w["w2a"][:],
                                     rhs=hid[0][:, c0:c0 + 512], start=True, stop=False)
                    nc.tensor.matmul(b_ps[:, sl], lhsT=w["w2b"][:],
                                     rhs=hid[1][:, c0:c0 + 512], start=False, stop=True)
                nc.scalar.activation(ben_sb[:, ch * 1024:(ch + 1) * 1024],
                                     b_ps[:], COPY, bias=float(b2_imm))
            nc.sync.dma_start(out=out[:], in_=ben_sb[:])

    nc.compile()
    return nc


def _chunk_idx(flat):
    # flat [n*128] -> [128, n] with idx[p, ch] = flat[ch*128 + p]
    return flat.reshape(-1, 128).T.copy()


def kernel(feats, query_idx, mv_idx, neighbor_idx,
           Wf, bf, Wi, bi, Wo, bo, Wmv, bmv, W1, b1, W2, b2):
    feats = np.ascontiguousarray(np.asarray(feats, dtype=np.float32))
    query_idx = np.asarray(query_idx).astype(np.int32)
    mv_idx = np.asarray(mv_idx).astype(np.int32)
    neighbor_idx = np.asarray(neighbor_idx).astype(np.int32)
    Wf, Wi, Wo = [np.asarray(x, np.float32) for x in (Wf, Wi, Wo)]
    Wmv, W1, W2 = [np.asarray(x, np.float32) for x in (Wmv, W1, W2)]
    bf, bi, bo, bmv, b1, b2 = [np.asarray(x, np.float32) for x in (bf, bi, bo, bmv, b1, b2)]

    b1_eff = b1 + W1.T @ np.concatenate([bmv, bmv])
    f16 = np.float16
    weights = {
        "wf": Wf.astype(f16), "wi": Wi.astype(f16), "wo": Wo.astype(f16),
        "wmva": Wmv[0:128].astype(f16), "wmvb": Wmv[128:256].astype(f16),
        "w1qa": W1[0:128, 0:128].astype(f16), "w1qb": W1[0:128, 128:256].astype(f16),
        "w1ma": W1[128:256, 0:128].astype(f16), "w1mb": W1[128:256, 128:256].astype(f16),
        "w2a": np.ascontiguousarray(W2[0:128]).astype(f16),
        "w2b": np.ascontiguousarray(W2[128:256]).astype(f16),
    }
    biases = {
        "bf": bf.reshape(128, 1), "bi": bi.reshape(128, 1), "bo": bo.reshape(128, 1),
        "b1a": b1_eff[0:128].reshape(128, 1).astype(np.float32),
        "b1b": b1_eff[128:256].reshape(128, 1).astype(np.float32),
    }

    in_maps = []
    for c in range(NCORES):
        b0 = c * BC
        flat_nbr = neighbor_idx[b0:b0 + BC].reshape(-1)  # [BC*KN], b-major
        cols = [_chunk_idx(flat_nbr),
                _chunk_idx(query_idx[b0:b0 + BC]),
                _chunk_idx(mv_idx[b0:b0 + BC])]
        idx_all = np.concatenate(cols, axis=1).astype(np.int32)
        im = {"feats": feats, "idx": np.ascontiguousarray(idx_all)}
        im.update(weights)
        im.update({k: np.ascontiguousarray(v) for k, v in biases.items()})
        in_maps.append(im)

    nc = _build(float(b2.reshape(-1)[0]))
    trace = bool(int(os.environ.get("KBENCH_TRACE", "0")))
    res = run_bass_kernel_spmd(nc, in_maps, core_ids=list(range(NCORES)), trace=trace)
    global LAST_EXEC_NS
    LAST_EXEC_NS = res.exec_time_ns
    outp = np.empty((B, 1), dtype=np.float32)
    for c in range(NCORES):
        outp[c * BC:(c + 1) * BC, 0] = res.results[c]["out"][0]
    return outp

